# revision 12
# baseline (speedup 1.0000x reference)
"""DenoiseNet loss kernel for 8 Trainium2 NeuronCores (Bass/Tile).

Strategy (hardcoded for the reference's shapes B=32, Nn=Nc=1024):
- Data-parallel over batch: each of 8 cores handles 4 batch elements
  (4096 points).  Activations live as [channels, points] so conv layers
  chain without transposes and BN stats are free-dim reductions.
- BatchNorm uses GLOBAL batch stats (the reference couples the whole
  batch): per-layer per-channel sum/sumsq are AllReduced across the 8
  cores (tiny payloads), then folded into one scalar-engine
  Relu(a*x+b) pass.  Conv biases are skipped: training-mode BN cancels
  them exactly.
- 1-NN + distance, gather-free: with x padded to [x, 1, |x|^2] and
  clean padded to [c, -|c|^2/2, -1/2], the K=5 matmul e[p,j] =
  -|x_p - c_j|^2 / 2.  argmin is vector-engine max over PSUM; the
  loss distance ||x_{m+1} - c_jmin||^2 is picked out of the NEXT
  module's e-matmul with a stored one-hot mask (uint8) via a fused
  scalar_tensor_tensor sum.  fp16 everywhere on the PE (same 10-bit
  mantissa as fp32r, full speed, half the SBUF).
- Output per core: sum over its batch of seed-weighted distances over
  all 4 modules.  Host sums across cores and divides by 32.
"""
import sys

sys.path.insert(0, "/opt/trn_rl_repo/concourse")
sys.path.insert(0, "/opt/trn_rl_repo")

import numpy as np

import concourse.bass as bass
import concourse.mybir as mybir
import concourse.tile as tile
from concourse import bacc
from concourse import bass_utils

N_CORES = 8
B, NN_PTS, NC_PTS = 32, 1024, 1024
BLOC = B // N_CORES          # 4 batch elements per core
PTS = BLOC * NN_PTS          # 4096 points per core
NCH, CH = 8, 512             # free-dim chunking for matmuls
NUM_MODULES = 4
LAYERS = [(3, 64), (64, 128), (128, 256), (256, 512), (512, 1024),
          (1024, 512), (512, 256)]   # BN+ReLU layers
NTOT = float(B * NN_PTS)     # 32768 points globally
BN_EPS = 1e-5

F32 = mybir.dt.float32
F16 = mybir.dt.float16
U8 = mybir.dt.uint8
ADD = mybir.AluOpType.add
MULT = mybir.AluOpType.mult
SUB = mybir.AluOpType.subtract
ISEQ = mybir.AluOpType.is_equal
AF = mybir.ActivationFunctionType

_cache = {}


def _cdiv(a, b):
    return (a + b - 1) // b


def _build_nc():
    nc = bacc.Bacc("TRN2", target_bir_lowering=False, debug=False,
                   enable_asserts=True, num_devices=N_CORES)

    # ---------------- DRAM I/O ----------------
    xaug_d = nc.dram_tensor("xaug", [5, PTS], F16, kind="ExternalInput").ap()
    caug_d = nc.dram_tensor("caug", [5, PTS], F16, kind="ExternalInput").ap()
    wcol_d = nc.dram_tensor("wcol", [128, 32], F32, kind="ExternalInput").ap()
    w_d, gb_d, b3_d = {}, {}, {}
    for m in range(NUM_MODULES):
        for l, (ci, co) in enumerate(LAYERS + [(256, 3)]):
            ci_p = min(ci, 128)
            nk = _cdiv(ci, 128)
            w_d[(m, l)] = nc.dram_tensor(
                f"w_{m}_{l}", [ci_p, nk * co], F16, kind="ExternalInput").ap()
            if l < 7:
                nt = _cdiv(co, 128)
                gb_d[(m, l)] = nc.dram_tensor(
                    f"gb_{m}_{l}", [128, 2 * nt], F32,
                    kind="ExternalInput").ap()
        b3_d[m] = nc.dram_tensor(f"b3_{m}", [3, 1], F32,
                                 kind="ExternalInput").ap()
    loss_d = nc.dram_tensor("loss", [1, 1], F32, kind="ExternalOutput").ap()

    with tile.TileContext(nc) as tc:
        with (
            tc.tile_pool(name="fixed", bufs=1) as fixed,
            tc.tile_pool(name="acts", bufs=13) as acts,
            tc.tile_pool(name="wts", bufs=36) as wts,
            tc.tile_pool(name="ohp", bufs=1) as ohp,
            tc.tile_pool(name="small", bufs=2) as small,
            tc.tile_pool(name="statsp", bufs=2) as statsp,
            tc.tile_pool(name="gbp", bufs=1) as gbp,
            tc.tile_pool(name="mlppsum", bufs=3, space="PSUM") as mlppsum,
            tc.tile_pool(name="knnpsum", bufs=2, space="PSUM") as knnpsum,
            tc.tile_pool(name="dpsum", bufs=1, space="PSUM") as dpsum,
            tc.tile_pool(name="dram", bufs=1, space="DRAM") as dram,
        ):
            # ---------------- persistent tiles ----------------
            x_aug = fixed.tile([5, PTS], F16, name="x_aug")
            G5 = fixed.tile([5, PTS], F16, name="G5")
            pred3 = fixed.tile([3, PTS], F16, name="pred3")
            wcol = fixed.tile([128, 32], F32, name="wcol")
            ones3 = fixed.tile([3, 1], F16, name="ones3")
            onescol = fixed.tile([128, 1], F32, name="onescol")
            epsc = fixed.tile([128, 1], F32, name="epsc")
            lossparts = fixed.tile([1, 4], F32, name="lossparts")
            losssum = fixed.tile([1, 1], F32, name="losssum")

            nc.sync.dma_start(x_aug[:], xaug_d[:])
            nc.sync.dma_start(G5[:], caug_d[:])
            nc.sync.dma_start(wcol[:], wcol_d[:])
            nc.gpsimd.memset(ones3[:], 1.0)
            nc.gpsimd.memset(onescol[:], 1.0)
            nc.gpsimd.memset(epsc[:], BN_EPS)
            nc.vector.memset(lossparts[:], 0.0)

            gb_sb = {}
            for m in range(NUM_MODULES):
                for l in range(7):
                    nt = _cdiv(LAYERS[l][1], 128)
                    t = gbp.tile([128, 2 * nt], F32, name=f"gb{m}_{l}",
                                 tag=f"gb{m}_{l}")
                    nc.sync.dma_start(t[:], gb_d[(m, l)][:])
                    gb_sb[(m, l)] = t
                t3 = gbp.tile([3, 1], F32, name=f"b3sb{m}", tag=f"b3{m}")
                nc.sync.dma_start(t3[:], b3_d[m][:])
                gb_sb[(m, 7)] = t3

            oh = {}
            for b in range(BLOC):
                for t in range(8):
                    oh[(b, t)] = ohp.tile([128, 1024], U8,
                                          name=f"oh{b}_{t}", tag=f"oh{b}_{t}")

            def knn_pass(m, update_oh=True, gather=True):
                """e5 matmuls over current x_aug: optionally gather module
                m-1's distances into d32, and refresh argmax one-hots."""
                d32 = None
                if gather:
                    d32 = small.tile([128, 32], F32, name=f"d32_{m}",
                                     tag="d32")
                for b in range(BLOC):
                    boff = b * NN_PTS
                    for t in range(8):
                        poff = boff + t * 128
                        eps = knnpsum.tile([128, 1024], F32,
                                           name=f"eps{m}_{b}_{t}", tag="eps")
                        lhsT = x_aug[0:5, poff:poff + 128]
                        nc.tensor.matmul(eps[:, 0:512], lhsT,
                                         G5[0:5, boff:boff + 512],
                                         start=True, stop=True)
                        nc.tensor.matmul(eps[:, 512:1024], lhsT,
                                         G5[0:5, boff + 512:boff + 1024],
                                         start=True, stop=True)
                        if gather:
                            scr = small.tile([128, 1024], F32,
                                             name=f"gs{m}_{b}_{t}",
                                             tag="gscr")
                            nc.vector.scalar_tensor_tensor(
                                out=scr[:], in0=eps[:], scalar=-2.0,
                                in1=oh[(b, t)][:], op0=MULT, op1=MULT,
                                accum_out=d32[:, 8 * b + t:8 * b + t + 1])
                        if update_oh:
                            mx8 = small.tile([128, 8], F32,
                                             name=f"mx{m}_{b}_{t}", tag="mx8")
                            nc.vector.max(mx8[:], eps[:])
                            nc.vector.tensor_scalar(
                                oh[(b, t)][:], eps[:], mx8[:, 0:1], None,
                                ISEQ)
                return d32

            def finalize_loss(d32, slot):
                """lossparts[slot] = sum_p sum_col wcol*d32."""
                wd = small.tile([128, 1], F32, name=f"wd{slot}", tag="wd")
                scr2 = small.tile([128, 32], F32, name=f"sc2_{slot}",
                                  tag="scr2")
                nc.vector.scalar_tensor_tensor(
                    out=scr2[:], in0=d32[:], scalar=1.0, in1=wcol[:],
                    op0=MULT, op1=MULT, accum_out=wd[:])
                pl = dpsum.tile([1, 1], F32, name=f"pl{slot}", tag="dps")
                nc.tensor.matmul(pl[:], wd[:], onescol[:],
                                 start=True, stop=True)
                nc.scalar.activation(lossparts[0:1, slot:slot + 1], pl[:],
                                     AF.Copy)

            # ---------------- modules ----------------
            for m in range(NUM_MODULES):
                # Phase K: e5(x_m): dist for m-1 + argmax/onehot for m
                d32 = knn_pass(m, update_oh=True, gather=(m > 0))
                if m > 0:
                    finalize_loss(d32, m - 1)

                # Phase M: PointFilterNet MLP on x_m
                xin = [x_aug[0:3, :]]
                for l, (ci, co) in enumerate(LAYERS):
                    ci_p, co_p = min(ci, 128), min(co, 128)
                    nk, nt = _cdiv(ci, 128), _cdiv(co, 128)
                    wks = []
                    for k in range(nk):
                        row = []
                        for mmw in range(nt):
                            cw = min(128, co - 128 * mmw)
                            wkm = wts.tile([ci_p, cw], F16,
                                           name=f"wl{m}_{l}_{k}_{mmw}",
                                           tag="wl")
                            off = k * co + mmw * 128
                            nc.sync.dma_start(
                                wkm[:], w_d[(m, l)][:, off:off + cw])
                            row.append(wkm)
                        wks.append(row)
                    xh, spart, sqpart = [], [], []
                    for mm in range(nt):
                        xh.append(acts.tile([co_p, PTS], F16,
                                            name=f"xh{m}_{l}_{mm}",
                                            tag="act"))
                        spart.append(statsp.tile([co_p, 8], F32, bufs=10,
                                                 name=f"sp{m}_{l}_{mm}",
                                                 tag="spart"))
                        sqpart.append(statsp.tile([co_p, 8], F32, bufs=10,
                                                  name=f"sq{m}_{l}_{mm}",
                                                  tag="sqpart"))
                    for n in range(NCH):
                        nsl = slice(n * CH, (n + 1) * CH)
                        for mm in range(nt):
                            ps = mlppsum.tile([co_p, CH], F32,
                                              name=f"ps{m}_{l}_{n}_{mm}",
                                              tag="ps")
                            for k in range(nk):
                                nc.tensor.matmul(ps[:], wks[k][mm][:],
                                                 xin[k][:, nsl],
                                                 start=(k == 0),
                                                 stop=(k == nk - 1))
                            # evict + chunk sum, alternating engines
                            if (n + mm) % 2 == 0:
                                nc.scalar.activation(
                                    xh[mm][:, nsl], ps[:], AF.Copy,
                                    accum_out=spart[mm][:, n:n + 1])
                            else:
                                nc.vector.tensor_scalar(
                                    xh[mm][:, nsl], ps[:], 1.0, 0.0,
                                    MULT, ADD,
                                    accum_out=spart[mm][:, n:n + 1])
                            # chunk sumsq, alternating DVE / gpsimd
                            sscr = small.tile([co_p, CH], F16,
                                              name=f"ss{m}_{l}_{n}_{mm}",
                                              tag="sscr", bufs=4)
                            nc.vector.scalar_tensor_tensor(
                                out=sscr[:], in0=xh[mm][:, nsl], scalar=1.0,
                                in1=xh[mm][:, nsl], op0=MULT, op1=MULT,
                                accum_out=sqpart[mm][:, n:n + 1])
                    # local -> staged stats
                    stats_loc = statsp.tile([128, 2 * nt], F32,
                                            name=f"stl{m}_{l}", tag="stl")
                    if co_p < 128:
                        nc.vector.memset(stats_loc[:], 0.0)
                    for mm in range(nt):
                        nc.vector.tensor_reduce(
                            stats_loc[0:co_p, mm:mm + 1], spart[mm][:],
                            axis=mybir.AxisListType.X, op=ADD)
                        nc.vector.tensor_reduce(
                            stats_loc[0:co_p, nt + mm:nt + mm + 1],
                            sqpart[mm][:], axis=mybir.AxisListType.X, op=ADD)
                    # global stats via AllReduce across the 8 cores
                    cc_in = dram.tile([128, 2 * nt], F32, name=f"cci{m}_{l}",
                                      tag=f"cci{m}_{l}")
                    cc_out = dram.tile([128, 2 * nt], F32, name=f"cco{m}_{l}",
                                       tag=f"cco{m}_{l}", addr_space="Shared")
                    nc.sync.dma_start(cc_in[:], stats_loc[:])
                    nc.gpsimd.collective_compute(
                        "AllReduce", ADD,
                        replica_groups=[list(range(N_CORES))],
                        ins=[cc_in.opt()], outs=[cc_out.opt()])
                    stats_g = statsp.tile([128, 2 * nt], F32,
                                          name=f"stg{m}_{l}", tag="stg")
                    nc.sync.dma_start(stats_g[:], cc_out[:])
                    # a = gamma*rstd ; b = beta - mean*a   (per channel)
                    mean = statsp.tile([128, nt], F32, name=f"mean{m}_{l}",
                                       tag="mean")
                    msq = statsp.tile([128, nt], F32, name=f"msq{m}_{l}",
                                      tag="msq")
                    var = statsp.tile([128, nt], F32, name=f"var{m}_{l}",
                                      tag="var")
                    sd = statsp.tile([128, nt], F32, name=f"sd{m}_{l}",
                                     tag="sd")
                    rinv = statsp.tile([128, nt], F32, name=f"rinv{m}_{l}",
                                       tag="rinv")
                    ab = statsp.tile([128, 2 * nt], F32, name=f"ab{m}_{l}",
                                     tag="ab")
                    gbt = gb_sb[(m, l)]
                    nc.vector.tensor_scalar(mean[:], stats_g[:, 0:nt],
                                            1.0 / NTOT, None, MULT)
                    nc.vector.scalar_tensor_tensor(
                        out=msq[:], in0=stats_g[:, 0:nt], scalar=1.0 / NTOT,
                        in1=mean[:], op0=MULT, op1=MULT)
                    nc.vector.scalar_tensor_tensor(
                        out=var[:], in0=stats_g[:, nt:2 * nt],
                        scalar=1.0 / NTOT, in1=msq[:], op0=MULT, op1=SUB)
                    nc.scalar.activation(sd[:], var[:], AF.Sqrt,
                                         bias=epsc[:, 0:1], scale=1.0)
                    nc.vector.reciprocal(rinv[:], sd[:])
                    nc.vector.tensor_tensor(ab[:, 0:nt], gbt[:, 0:nt],
                                            rinv[:], MULT)
                    nc.vector.scalar_tensor_tensor(
                        out=ab[:, nt:2 * nt], in0=mean[:], scalar=-1.0,
                        in1=ab[:, 0:nt], op0=MULT, op1=MULT)
                    nc.vector.tensor_tensor(ab[:, nt:2 * nt],
                                            gbt[:, nt:2 * nt],
                                            ab[:, nt:2 * nt], ADD)
                    # BN apply + ReLU (one ACT pass per co-tile)
                    xps = []
                    for mm in range(nt):
                        xp = acts.tile([co_p, PTS], F16,
                                       name=f"xp{m}_{l}_{mm}", tag="act")
                        nc.scalar.activation(
                            xp[:], xh[mm][:], AF.Relu,
                            bias=ab[0:co_p, nt + mm:nt + mm + 1],
                            scale=ab[0:co_p, mm:mm + 1])
                        xps.append(xp)
                    xin = xps
                # fc3 -> tanh -> pred3
                wk3 = []
                for k in range(2):
                    wk = wts.tile([128, 3], F16, name=f"wl{m}_7_{k}",
                                  tag="wl")
                    nc.sync.dma_start(wk[:], w_d[(m, 7)][:, k * 3:k * 3 + 3])
                    wk3.append(wk)
                for n in range(NCH):
                    nsl = slice(n * CH, (n + 1) * CH)
                    ps3 = mlppsum.tile([3, CH], F32, name=f"ps3{m}_{n}",
                                       tag="ps")
                    for k in range(2):
                        nc.tensor.matmul(ps3[:], wk3[k][:], xin[k][:, nsl],
                                         start=(k == 0), stop=(k == 1))
                    nc.scalar.activation(pred3[0:3, nsl], ps3[:], AF.Tanh,
                                         bias=gb_sb[(m, 7)][0:3, 0:1],
                                         scale=1.0)

                # Phase D: x += pred ; refresh x^2 row
                nc.gpsimd.tensor_tensor(x_aug[0:3, :], x_aug[0:3, :],
                                        pred3[0:3, :], ADD)
                nc.gpsimd.tensor_tensor(pred3[:], x_aug[0:3, :],
                                        x_aug[0:3, :], MULT)
                for n in range(NCH):
                    nsl = slice(n * CH, (n + 1) * CH)
                    psq = dpsum.tile([1, CH], F32, name=f"psq{m}_{n}",
                                     tag="dps")
                    nc.tensor.matmul(psq[:], ones3[:], pred3[0:3, nsl],
                                     start=True, stop=True)
                    # engine writes must start at a 32-aligned partition, so
                    # stage the x^2 row at partition 0 and DMA it to row 4
                    xrow = small.tile([1, CH], F16, name=f"xr{m}_{n}",
                                      tag="xrow", bufs=3)
                    nc.scalar.activation(xrow[:], psq[:], AF.Copy)
                    nc.sync.dma_start(x_aug[4:5, nsl], xrow[:])

            # final e5(x_4) pass: distances for module 3
            d32 = knn_pass(NUM_MODULES, update_oh=False, gather=True)
            finalize_loss(d32, NUM_MODULES - 1)

            nc.vector.tensor_reduce(losssum[:], lossparts[:],
                                    axis=mybir.AxisListType.X, op=ADD)
            nc.sync.dma_start(loss_d[:], losssum[:])

    nc.compile()
    return nc


def _host_prep(pcl_noisy, pcl_clean, pcl_seeds, params):
    """Build the per-core input maps (numpy only)."""
    pcl_noisy = np.asarray(pcl_noisy, np.float32)
    pcl_clean = np.asarray(pcl_clean, np.float32)
    pcl_seeds = np.asarray(pcl_seeds, np.float32)

    # seed weights (exactly the reference formula, fp32)
    sd = ((pcl_noisy - pcl_seeds) ** 2).sum(-1).astype(np.float32)  # [B, Nn]
    mx = sd[:, -1:]
    w = np.exp(-sd / (mx / np.float32(9.0))).astype(np.float32)
    w = (w / w.sum(axis=1, keepdims=True)).astype(np.float32)       # [B, Nn]

    xc = (pcl_noisy - pcl_seeds).astype(np.float32)   # [B, Nn, 3]
    cc = (pcl_clean - pcl_seeds).astype(np.float32)   # [B, Nc, 3]
    csq = (cc ** 2).sum(-1).astype(np.float32)        # [B, Nc]

    shared = {}
    for m, p in enumerate(params):
        layer_params = list(p["enc"]) + [p["fc1"], p["fc2"]]
        for l, lp in enumerate(layer_params):
            W = np.asarray(lp[0], np.float32)
            g = np.asarray(lp[2], np.float32)
            be = np.asarray(lp[3], np.float32)
            ci, co = W.shape[1], W.shape[0]
            nk, nt = _cdiv(ci, 128), _cdiv(co, 128)
            WT = np.ascontiguousarray(W.T)            # [ci, co]
            Wp = np.concatenate([WT[128 * k:min(128 * (k + 1), ci)]
                                 for k in range(nk)], axis=1)
            shared[f"w_{m}_{l}"] = Wp.astype(np.float16)
            gb = np.zeros((128, 2 * nt), np.float32)
            for mm in range(nt):
                seg = slice(128 * mm, min(128 * (mm + 1), co))
                npart = seg.stop - seg.start
                gb[0:npart, mm] = g[seg]
                gb[0:npart, nt + mm] = be[seg]
            shared[f"gb_{m}_{l}"] = gb
        W3, b3 = p["fc3"]
        W3 = np.asarray(W3, np.float32)               # [3, 256]
        WT3 = np.ascontiguousarray(W3.T)              # [256, 3]
        Wp3 = np.concatenate([WT3[0:128], WT3[128:256]], axis=1)  # [128, 6]
        shared[f"w_{m}_7"] = Wp3.astype(np.float16)
        shared[f"b3_{m}"] = np.asarray(b3, np.float32).reshape(3, 1)

    in_maps = []
    for c in range(N_CORES):
        bs = [c * BLOC + b for b in range(BLOC)]
        xT = np.concatenate([xc[b].T for b in bs], axis=1)     # [3, PTS]
        cT = np.concatenate([cc[b].T for b in bs], axis=1)     # [3, PTS]
        xaug = np.concatenate(
            [xT, np.ones((1, PTS), np.float32),
             (xT ** 2).sum(0, keepdims=True)], axis=0)         # [5, PTS]
        caug = np.concatenate(
            [cT, np.concatenate([-csq[b][None, :] / 2.0 for b in bs], axis=1),
             np.full((1, PTS), -0.5, np.float32)], axis=0)     # [5, PTS]
        wc = np.zeros((128, 32), np.float32)
        for bi, b in enumerate(bs):
            for t in range(8):
                wc[:, 8 * bi + t] = w[b, t * 128:(t + 1) * 128]
        m = {
            "xaug": xaug.astype(np.float16),
            "caug": caug.astype(np.float16),
            "wcol": wc,
        }
        m.update(shared)
        in_maps.append(m)
    return in_maps


def kernel(pcl_noisy, pcl_clean, pcl_seeds, pcl_std, params):
    if "nc" not in _cache:
        _cache["nc"] = _build_nc()
    nc = _cache["nc"]
    in_maps = _host_prep(pcl_noisy, pcl_clean, pcl_seeds, params)
    res = bass_utils.run_bass_kernel_spmd(
        nc, in_maps, core_ids=list(range(N_CORES)))
    total = np.float32(0.0)
    for c in range(N_CORES):
        total += res.results[c]["loss"][0, 0]
    return np.asarray(total / np.float32(B), dtype=np.float32)
